# revision 12
# baseline (speedup 1.0000x reference)
"""Cost-sensitive focal NLL loss on 8 Trainium2 NeuronCores.

Computes, for feature [N, C] logits and label [N] int:
    log_p = log_softmax(feature, axis=1)
    p = exp(log_p); beta = (1 - p)**2
    counts = bincount(label, C); ni = counts[label]; r = ni / N
    alpha = exp(r - 1) / r
    loss = -mean(alpha * beta[i, label[i]] * log_p[i, label[i]])

Only the label-column of log_p/beta is needed, so each core streams its
batch shard once (1 MiB DMAs, bf16 -- the 2e-2 tolerance dwarfs the
~2e-5 it costs), exponentiates on ScalarE and row-reduces on VectorE.
Everything that depends only on `label` (class histogram, gather
offsets -> the label logits themselves, per-row counts) is integer
bookkeeping prepared host-side during sharding; the device computes all
the float math: sumexp over [N, C], log-softmax at the label, focal
beta, cost weight alpha, and one partial sum per core.  The host adds
the 8 partials and divides by -N.
"""

import os

import numpy as np

import concourse.bacc as bacc
import concourse.mybir as mybir
import concourse.tile as tile
from concourse.bass_utils import run_bass_kernel_spmd

N_CORES = 8
N = 16384
C = 1000
P = 128
ROWS = N // N_CORES          # 2048 rows per core
T = ROWS // P                # 16 row-tiles per core
# row-tiles per DMA: small transfers first so ScalarE starts early, then
# 1 MiB quads for bandwidth
DMA_GROUPS = (1, 1, 2, 4, 4, 4)
assert sum(DMA_GROUPS) == T

FP = mybir.dt.float32
BF = mybir.dt.bfloat16

LAST_RESULTS = None  # BassKernelResults of the most recent run (for profiling)


def build_program(dump_debug: bool = False):
    nc = bacc.Bacc(
        "TRN2",
        target_bir_lowering=False,
        debug=False,
        enable_asserts=False,
        num_devices=N_CORES,
    )

    feature = nc.dram_tensor("feature", [ROWS, C], BF, kind="ExternalInput")
    xg_cm = nc.dram_tensor("xg_cm", [P, T], FP, kind="ExternalInput")
    cnt_cm = nc.dram_tensor("cnt_cm", [P, T], FP, kind="ExternalInput")
    out = nc.dram_tensor("out", [1, 1], FP, kind="ExternalOutput")
    dbg = {}
    if dump_debug:
        for nm in ("d_s", "d_u", "d_alpha"):
            dbg[nm] = nc.dram_tensor(nm, [P, T], FP, kind="ExternalOutput")

    with tile.TileContext(nc) as tc:
        with (
            tc.tile_pool(name="const", bufs=1) as const_pool,
            tc.tile_pool(name="feat", bufs=4) as feat_pool,
            tc.tile_pool(name="escr", bufs=3) as escr_pool,
            tc.tile_pool(name="small", bufs=1) as small_pool,
        ):
            neg1_col = const_pool.tile([P, 1], FP)
            nc.vector.memset(neg1_col[:], -1.0)

            # small inputs via the gpsimd SWDGE queue (idle during the whole
            # stream) so neither the sync queue nor ScalarE pays the dispatch
            xg = small_pool.tile([P, T], FP)
            nc.gpsimd.dma_start(xg[:], xg_cm.ap())
            cnt = small_pool.tile([P, T], FP)
            nc.gpsimd.dma_start(cnt[:], cnt_cm.ap())

            # per-row alpha = exp(r - 1)/r, r = cnt/N (no Ln needed); on
            # ScalarE this also pulls the Exp table load into the DMA wait
            e1 = small_pool.tile([P, T], FP)
            nc.scalar.activation(
                e1[:],
                cnt[:],
                mybir.ActivationFunctionType.Exp,
                bias=neg1_col[:],
                scale=1.0 / N,
            )
            rc = small_pool.tile([P, T], FP)
            nc.vector.reciprocal(rc[:], cnt[:])
            alpha = small_pool.tile([P, T], FP)  # exp(r-1) * N * (1/cnt)
            nc.vector.scalar_tensor_tensor(
                alpha[:],
                in0=e1[:],
                scalar=float(N),
                in1=rc[:],
                op0=mybir.AluOpType.mult,
                op1=mybir.AluOpType.mult,
            )
            # exp of the gathered label logits (input lands early, so this
            # can never stall the stream wherever the scheduler slots it)
            ex = small_pool.tile([P, T], FP)
            nc.scalar.activation(ex[:], xg[:], mybir.ActivationFunctionType.Exp)

            # ---- stream feature tiles: s[row] = sum_c exp(feature[row, c])
            # One ACT exp per landed DMA group; per-row sums as per-tile 2-D
            # [P, C] bf16 reduces on VectorE -- 2-D with a free_size-1 fp32
            # output keeps every non-scalar operand 2-byte/packed, which is
            # what unlocks the DVE 2x/4x fast modes.
            feat_t = feature.ap().rearrange("(t p) c -> p t c", p=P)
            s_col = small_pool.tile([P, T], FP)
            t0 = 0
            for g in DMA_GROUPS:
                ft = feat_pool.tile([P, g * C], BF, name="ft")
                nc.sync.dma_start(
                    ft[:].rearrange("p (g c) -> p g c", g=g),
                    feat_t[:, t0 : t0 + g, :],
                )
                esc = escr_pool.tile([P, g * C], BF, name="esc")
                nc.scalar.activation(
                    esc[:], ft[:], mybir.ActivationFunctionType.Exp
                )
                for j in range(g):
                    nc.vector.tensor_reduce(
                        s_col[:, t0 + j : t0 + j + 1],
                        esc[:, j * C : (j + 1) * C],
                        axis=mybir.AxisListType.X,
                        op=mybir.AluOpType.add,
                    )
                t0 += g

            # ---- per-row tail ----
            lse = small_pool.tile([P, T], FP)
            nc.scalar.activation(lse[:], s_col[:], mybir.ActivationFunctionType.Ln)
            rs = small_pool.tile([P, T], FP)
            nc.vector.reciprocal(rs[:], s_col[:])

            logp = small_pool.tile([P, T], FP)
            nc.vector.tensor_tensor(
                logp[:], xg[:], lse[:], op=mybir.AluOpType.subtract
            )
            pp = small_pool.tile([P, T], FP)  # p = exp(x)/s
            nc.vector.tensor_tensor(pp[:], ex[:], rs[:], op=mybir.AluOpType.mult)

            # u = (p-1)^2 * logp * alpha  ==  ((p-1)*logp) * ((p-1)*alpha)
            t1 = small_pool.tile([P, T], FP)
            nc.vector.scalar_tensor_tensor(
                t1[:],
                in0=pp[:],
                scalar=-1.0,
                in1=logp[:],
                op0=mybir.AluOpType.add,
                op1=mybir.AluOpType.mult,
            )
            t2 = small_pool.tile([P, T], FP)
            nc.vector.scalar_tensor_tensor(
                t2[:],
                in0=pp[:],
                scalar=-1.0,
                in1=alpha[:],
                op0=mybir.AluOpType.add,
                op1=mybir.AluOpType.mult,
            )
            u = small_pool.tile([P, T], FP)
            nc.vector.tensor_tensor(u[:], t1[:], t2[:], op=mybir.AluOpType.mult)

            # partial = sum_{p,t} u  (row-reduce on DVE, cross-partition on
            # GpSimd -- keeps the TensorEngine entirely out of the program)
            us = small_pool.tile([P, 1], FP)
            nc.vector.tensor_reduce(
                us[:], u[:], axis=mybir.AxisListType.X, op=mybir.AluOpType.add
            )
            fin_sb = small_pool.tile([1, 1], FP)
            nc.gpsimd.tensor_reduce(
                fin_sb[:], us[:], axis=mybir.AxisListType.C, op=mybir.AluOpType.add
            )
            nc.sync.dma_start(out.ap(), fin_sb[:])

            if dump_debug:
                nc.sync.dma_start(dbg["d_s"].ap(), s_col[:])
                nc.sync.dma_start(dbg["d_u"].ap(), u[:])
                nc.sync.dma_start(dbg["d_alpha"].ap(), alpha[:])

    nc.compile()
    return nc


_NC_CACHE = None


def _get_nc():
    global _NC_CACHE
    if _NC_CACHE is None:
        _NC_CACHE = build_program()
    return _NC_CACHE


def _to_bf16(a: np.ndarray) -> np.ndarray:
    """fp32 -> bf16 (round-to-nearest-even) as a uint16 view numpy array."""
    try:
        import ml_dtypes

        return a.astype(ml_dtypes.bfloat16)
    except ImportError:
        b = a.view(np.uint32)
        rounded = (b + 0x7FFF + ((b >> 16) & 1)) >> 16
        return rounded.astype(np.uint16).view(np.dtype("V2"))


def kernel(feature: np.ndarray, label: np.ndarray) -> np.ndarray:
    global LAST_RESULTS
    feature = np.ascontiguousarray(np.asarray(feature, dtype=np.float32))
    label = np.asarray(label)
    assert feature.shape == (N, C), feature.shape
    assert label.shape == (N,), label.shape

    lab32 = label.astype(np.int32)
    counts = np.bincount(lab32, minlength=C).astype(np.float32)  # global
    picked = feature[np.arange(N), lab32]  # label logits, fp32

    in_maps = []
    for k in range(N_CORES):
        fshard = feature[k * ROWS : (k + 1) * ROWS]
        lshard = lab32[k * ROWS : (k + 1) * ROWS]
        # column-major: [p, t] = row t*P + p, matching row-tile partitions
        lab_cm = lshard.reshape(T, P).T
        xg_cm = picked[k * ROWS : (k + 1) * ROWS].reshape(T, P).T
        in_maps.append(
            {
                "feature": np.ascontiguousarray(_to_bf16(fshard)),
                "xg_cm": np.ascontiguousarray(xg_cm),
                "cnt_cm": np.ascontiguousarray(counts[lab_cm]),
            }
        )

    nc = _get_nc()
    trace = bool(int(os.environ.get("KERNEL_TRACE", "0")))
    res = run_bass_kernel_spmd(
        nc,
        in_maps,
        core_ids=list(range(N_CORES)),
        trace=trace,
    )
    LAST_RESULTS = res

    total = 0.0
    for k in range(N_CORES):
        total += float(res.results[k]["out"][0, 0])
    return np.float32(-total / N)


# revision 14
# speedup vs baseline: 1.0272x; 1.0272x over previous
"""Cost-sensitive focal NLL loss on 8 Trainium2 NeuronCores.

Computes, for feature [N, C] logits and label [N] int:
    log_p = log_softmax(feature, axis=1)
    p = exp(log_p); beta = (1 - p)**2
    counts = bincount(label, C); ni = counts[label]; r = ni / N
    alpha = exp(r - 1) / r
    loss = -mean(alpha * beta[i, label[i]] * log_p[i, label[i]])

Only the label-column of log_p/beta is needed, so each core streams its
batch shard once (1 MiB DMAs, bf16 -- the 2e-2 tolerance dwarfs the
~2e-5 it costs), exponentiates on ScalarE and row-reduces on VectorE.
Everything that depends only on `label` (class histogram, gather
offsets -> the label logits themselves, per-row counts) is integer
bookkeeping prepared host-side during sharding; the device computes all
the float math: sumexp over [N, C], log-softmax at the label, focal
beta, cost weight alpha, and one partial sum per core.  The host adds
the 8 partials and divides by -N.
"""

import os

import numpy as np

import concourse.bacc as bacc
import concourse.mybir as mybir
import concourse.tile as tile
from concourse.bass_utils import run_bass_kernel_spmd

N_CORES = 8
N = 16384
C = 1000
P = 128
ROWS = N // N_CORES          # 2048 rows per core
T = ROWS // P                # 16 row-tiles per core
# row-tiles per DMA: small transfers first so ScalarE starts early, then
# 1 MiB quads for bandwidth, then single tiles whose row-sums ride the exp
# as ScalarE accum_out (balances the reduce load across both engines and
# leaves VectorE nothing to trail on at stream end)
DMA_GROUPS = (1, 1, 2, 4, 4, 1, 1, 1, 1)
N_ACCUM_TILES = 4            # trailing 1-tile groups summed via accum_out
assert sum(DMA_GROUPS) == T

FP = mybir.dt.float32
BF = mybir.dt.bfloat16

LAST_RESULTS = None  # BassKernelResults of the most recent run (for profiling)


def build_program(dump_debug: bool = False):
    nc = bacc.Bacc(
        "TRN2",
        target_bir_lowering=False,
        debug=False,
        enable_asserts=False,
        num_devices=N_CORES,
    )

    feature = nc.dram_tensor("feature", [ROWS, C], BF, kind="ExternalInput")
    xg_cm = nc.dram_tensor("xg_cm", [P, T], FP, kind="ExternalInput")
    cnt_cm = nc.dram_tensor("cnt_cm", [P, T], FP, kind="ExternalInput")
    out = nc.dram_tensor("out", [1, 1], FP, kind="ExternalOutput")
    dbg = {}
    if dump_debug:
        for nm in ("d_s", "d_u", "d_alpha"):
            dbg[nm] = nc.dram_tensor(nm, [P, T], FP, kind="ExternalOutput")

    with tile.TileContext(nc) as tc:
        with (
            tc.tile_pool(name="const", bufs=1) as const_pool,
            tc.tile_pool(name="feat", bufs=4) as feat_pool,
            tc.tile_pool(name="escr", bufs=3) as escr_pool,
            tc.tile_pool(name="small", bufs=1) as small_pool,
        ):
            neg1_col = const_pool.tile([P, 1], FP)
            nc.vector.memset(neg1_col[:], -1.0)

            # small inputs via the gpsimd SWDGE queue (idle during the whole
            # stream) so neither the sync queue nor ScalarE pays the dispatch
            xg = small_pool.tile([P, T], FP)
            nc.gpsimd.dma_start(xg[:], xg_cm.ap())
            cnt = small_pool.tile([P, T], FP)
            nc.gpsimd.dma_start(cnt[:], cnt_cm.ap())

            # ---- stream feature tiles: s[row] = sum_c exp(feature[row, c])
            # One ACT exp per landed DMA group.  Row-sums: per-tile [P, C]
            # reduces on VectorE for the leading groups, fused accum_out on
            # ScalarE for the trailing single-tile groups.
            feat_t = feature.ap().rearrange("(t p) c -> p t c", p=P)
            s_col = small_pool.tile([P, T], FP)
            t0 = 0
            for gi, g in enumerate(DMA_GROUPS):
                ft = feat_pool.tile([P, g * C], BF, name="ft")
                nc.sync.dma_start(
                    ft[:].rearrange("p (g c) -> p g c", g=g),
                    feat_t[:, t0 : t0 + g, :],
                )
                esc = escr_pool.tile([P, g * C], BF, name="esc")
                accum = t0 >= T - N_ACCUM_TILES
                nc.scalar.activation(
                    esc[:],
                    ft[:],
                    mybir.ActivationFunctionType.Exp,
                    accum_out=s_col[:, t0 : t0 + 1] if accum else None,
                )
                if not accum:
                    for j in range(g):
                        nc.vector.tensor_reduce(
                            s_col[:, t0 + j : t0 + j + 1],
                            esc[:, j * C : (j + 1) * C],
                            axis=mybir.AxisListType.X,
                            op=mybir.AluOpType.add,
                        )
                t0 += g

                if gi == 2:
                    # per-row alpha = exp(r-1)/r, r = cnt/N (no Ln needed);
                    # emitted mid-stream: its inputs landed long ago, and this
                    # placement keeps the scheduler from parking it in front
                    # of the first stream exp on ScalarE
                    e1 = small_pool.tile([P, T], FP)
                    nc.scalar.activation(
                        e1[:],
                        cnt[:],
                        mybir.ActivationFunctionType.Exp,
                        bias=neg1_col[:],
                        scale=1.0 / N,
                    )
                    rc = small_pool.tile([P, T], FP)
                    nc.vector.reciprocal(rc[:], cnt[:])
                    alpha = small_pool.tile([P, T], FP)  # exp(r-1)*N*(1/cnt)
                    nc.vector.scalar_tensor_tensor(
                        alpha[:],
                        in0=e1[:],
                        scalar=float(N),
                        in1=rc[:],
                        op0=mybir.AluOpType.mult,
                        op1=mybir.AluOpType.mult,
                    )
                    ex = small_pool.tile([P, T], FP)
                    nc.scalar.activation(
                        ex[:], xg[:], mybir.ActivationFunctionType.Exp
                    )

            # ---- per-row tail ----
            lse = small_pool.tile([P, T], FP)
            nc.scalar.activation(lse[:], s_col[:], mybir.ActivationFunctionType.Ln)
            rs = small_pool.tile([P, T], FP)
            nc.vector.reciprocal(rs[:], s_col[:])

            logp = small_pool.tile([P, T], FP)
            nc.vector.tensor_tensor(
                logp[:], xg[:], lse[:], op=mybir.AluOpType.subtract
            )
            pp = small_pool.tile([P, T], FP)  # p = exp(x)/s
            nc.vector.tensor_tensor(pp[:], ex[:], rs[:], op=mybir.AluOpType.mult)

            # u = (p-1)^2 * logp * alpha  ==  ((p-1)*logp) * ((p-1)*alpha)
            t1 = small_pool.tile([P, T], FP)
            nc.vector.scalar_tensor_tensor(
                t1[:],
                in0=pp[:],
                scalar=-1.0,
                in1=logp[:],
                op0=mybir.AluOpType.add,
                op1=mybir.AluOpType.mult,
            )
            t2 = small_pool.tile([P, T], FP)
            nc.vector.scalar_tensor_tensor(
                t2[:],
                in0=pp[:],
                scalar=-1.0,
                in1=alpha[:],
                op0=mybir.AluOpType.add,
                op1=mybir.AluOpType.mult,
            )
            u = small_pool.tile([P, T], FP)
            nc.vector.tensor_tensor(u[:], t1[:], t2[:], op=mybir.AluOpType.mult)

            # partial = sum_{p,t} u  (row-reduce on DVE, cross-partition on
            # GpSimd -- keeps the TensorEngine entirely out of the program)
            us = small_pool.tile([P, 1], FP)
            nc.vector.tensor_reduce(
                us[:], u[:], axis=mybir.AxisListType.X, op=mybir.AluOpType.add
            )
            fin_sb = small_pool.tile([1, 1], FP)
            nc.gpsimd.tensor_reduce(
                fin_sb[:], us[:], axis=mybir.AxisListType.C, op=mybir.AluOpType.add
            )
            nc.sync.dma_start(out.ap(), fin_sb[:])

            if dump_debug:
                nc.sync.dma_start(dbg["d_s"].ap(), s_col[:])
                nc.sync.dma_start(dbg["d_u"].ap(), u[:])
                nc.sync.dma_start(dbg["d_alpha"].ap(), alpha[:])

    nc.compile()
    return nc


_NC_CACHE = None


def _get_nc():
    global _NC_CACHE
    if _NC_CACHE is None:
        _NC_CACHE = build_program()
    return _NC_CACHE


def _to_bf16(a: np.ndarray) -> np.ndarray:
    """fp32 -> bf16 (round-to-nearest-even) as a uint16 view numpy array."""
    try:
        import ml_dtypes

        return a.astype(ml_dtypes.bfloat16)
    except ImportError:
        b = a.view(np.uint32)
        rounded = (b + 0x7FFF + ((b >> 16) & 1)) >> 16
        return rounded.astype(np.uint16).view(np.dtype("V2"))


def kernel(feature: np.ndarray, label: np.ndarray) -> np.ndarray:
    global LAST_RESULTS
    feature = np.ascontiguousarray(np.asarray(feature, dtype=np.float32))
    label = np.asarray(label)
    assert feature.shape == (N, C), feature.shape
    assert label.shape == (N,), label.shape

    lab32 = label.astype(np.int32)
    counts = np.bincount(lab32, minlength=C).astype(np.float32)  # global
    picked = feature[np.arange(N), lab32]  # label logits, fp32

    in_maps = []
    for k in range(N_CORES):
        fshard = feature[k * ROWS : (k + 1) * ROWS]
        lshard = lab32[k * ROWS : (k + 1) * ROWS]
        # column-major: [p, t] = row t*P + p, matching row-tile partitions
        lab_cm = lshard.reshape(T, P).T
        xg_cm = picked[k * ROWS : (k + 1) * ROWS].reshape(T, P).T
        in_maps.append(
            {
                "feature": np.ascontiguousarray(_to_bf16(fshard)),
                "xg_cm": np.ascontiguousarray(xg_cm),
                "cnt_cm": np.ascontiguousarray(counts[lab_cm]),
            }
        )

    nc = _get_nc()
    trace = bool(int(os.environ.get("KERNEL_TRACE", "0")))
    res = run_bass_kernel_spmd(
        nc,
        in_maps,
        core_ids=list(range(N_CORES)),
        trace=trace,
    )
    LAST_RESULTS = res

    total = 0.0
    for k in range(N_CORES):
        total += float(res.results[k]["out"][0, 0])
    return np.float32(-total / N)


# revision 17
# speedup vs baseline: 1.0493x; 1.0215x over previous
"""Cost-sensitive focal NLL loss on 8 Trainium2 NeuronCores.

Computes, for feature [N, C] logits and label [N] int:
    log_p = log_softmax(feature, axis=1)
    p = exp(log_p); beta = (1 - p)**2
    counts = bincount(label, C); ni = counts[label]; r = ni / N
    alpha = exp(r - 1) / r
    loss = -mean(alpha * beta[i, label[i]] * log_p[i, label[i]])

Only the label-column of log_p/beta is needed, so each core streams its
batch shard once (1 MiB DMAs, bf16 -- the 2e-2 tolerance dwarfs the
~2e-5 it costs), exponentiates on ScalarE and row-reduces on VectorE.
Everything that depends only on `label` (class histogram, gather
offsets -> the label logits themselves, per-row counts) is integer
bookkeeping prepared host-side during sharding; the device computes all
the float math: sumexp over [N, C], log-softmax at the label, focal
beta, cost weight alpha, and one partial sum per core.  The host adds
the 8 partials and divides by -N.
"""

import os

import numpy as np

import concourse.bacc as bacc
import concourse.mybir as mybir
import concourse.tile as tile
from concourse.bass_utils import run_bass_kernel_spmd

N_CORES = 8
N = 16384
C = 1000
P = 128
ROWS = N // N_CORES          # 2048 rows per core
T = ROWS // P                # 16 row-tiles per core
# row-tiles per DMA: small transfers first so ScalarE starts early, then
# uniform 0.5 MiB pairs that land faster than ScalarE consumes them (no
# mid-stream gaps from big-transfer completion latency).  Tile 14 sums via
# ScalarE accum_out to offload VectorE; the last tile reduces on VectorE so
# that reduce overlaps the Ln table load on ScalarE.
DMA_GROUPS = (1, 1, 2, 2, 2, 2, 2, 2, 1, 1)
ACCUM_TILES = (14,)          # row-sum via fused accum_out on ScalarE
assert sum(DMA_GROUPS) == T

FP = mybir.dt.float32
BF = mybir.dt.bfloat16

LAST_RESULTS = None  # BassKernelResults of the most recent run (for profiling)


def build_program(dump_debug: bool = False):
    nc = bacc.Bacc(
        "TRN2",
        target_bir_lowering=False,
        debug=False,
        enable_asserts=False,
        num_devices=N_CORES,
    )

    feature = nc.dram_tensor("feature", [ROWS, C], BF, kind="ExternalInput")
    xg_cm = nc.dram_tensor("xg_cm", [P, T], FP, kind="ExternalInput")
    cnt_cm = nc.dram_tensor("cnt_cm", [P, T], FP, kind="ExternalInput")
    out = nc.dram_tensor("out", [1, 1], FP, kind="ExternalOutput")
    dbg = {}
    if dump_debug:
        for nm in ("d_s", "d_u", "d_alpha"):
            dbg[nm] = nc.dram_tensor(nm, [P, T], FP, kind="ExternalOutput")

    with tile.TileContext(nc) as tc:
        with (
            tc.tile_pool(name="const", bufs=1) as const_pool,
            tc.tile_pool(name="feat", bufs=6) as feat_pool,
            tc.tile_pool(name="escr", bufs=4) as escr_pool,
            tc.tile_pool(name="small", bufs=1) as small_pool,
        ):
            neg1_col = const_pool.tile([P, 1], FP)
            nc.vector.memset(neg1_col[:], -1.0)

            # small inputs via the gpsimd SWDGE queue (idle during the whole
            # stream) so neither the sync queue nor ScalarE pays the dispatch
            xg = small_pool.tile([P, T], FP)
            nc.gpsimd.dma_start(xg[:], xg_cm.ap())
            cnt = small_pool.tile([P, T], FP)
            nc.gpsimd.dma_start(cnt[:], cnt_cm.ap())

            # ---- stream feature tiles: s[row] = sum_c exp(feature[row, c])
            # One ACT exp per landed DMA group.  Row-sums: per-tile [P, C]
            # reduces on VectorE for the leading groups, fused accum_out on
            # ScalarE for the trailing single-tile groups.
            feat_t = feature.ap().rearrange("(t p) c -> p t c", p=P)
            s_col = small_pool.tile([P, T], FP)
            t0 = 0
            for gi, g in enumerate(DMA_GROUPS):
                ft = feat_pool.tile([P, g * C], BF, name="ft")
                nc.sync.dma_start(
                    ft[:].rearrange("p (g c) -> p g c", g=g),
                    feat_t[:, t0 : t0 + g, :],
                )
                esc = escr_pool.tile([P, g * C], BF, name="esc")
                accum = t0 in ACCUM_TILES
                nc.scalar.activation(
                    esc[:],
                    ft[:],
                    mybir.ActivationFunctionType.Exp,
                    accum_out=s_col[:, t0 : t0 + 1] if accum else None,
                )
                if not accum:
                    for j in range(g):
                        nc.vector.tensor_reduce(
                            s_col[:, t0 + j : t0 + j + 1],
                            esc[:, j * C : (j + 1) * C],
                            axis=mybir.AxisListType.X,
                            op=mybir.AluOpType.add,
                        )
                t0 += g

                if gi == 2:
                    # per-row alpha = exp(r-1)/r, r = cnt/N (no Ln needed);
                    # emitted mid-stream: its inputs landed long ago, and this
                    # placement keeps the scheduler from parking it in front
                    # of the first stream exp on ScalarE
                    e1 = small_pool.tile([P, T], FP)
                    nc.scalar.activation(
                        e1[:],
                        cnt[:],
                        mybir.ActivationFunctionType.Exp,
                        bias=neg1_col[:],
                        scale=1.0 / N,
                    )
                    rc = small_pool.tile([P, T], FP)
                    nc.vector.reciprocal(rc[:], cnt[:])
                    alpha = small_pool.tile([P, T], FP)  # exp(r-1)*N*(1/cnt)
                    nc.vector.scalar_tensor_tensor(
                        alpha[:],
                        in0=e1[:],
                        scalar=float(N),
                        in1=rc[:],
                        op0=mybir.AluOpType.mult,
                        op1=mybir.AluOpType.mult,
                    )
                    ex = small_pool.tile([P, T], FP)
                    nc.scalar.activation(
                        ex[:], xg[:], mybir.ActivationFunctionType.Exp
                    )

            # ---- per-row tail ----
            lse = small_pool.tile([P, T], FP)
            nc.scalar.activation(lse[:], s_col[:], mybir.ActivationFunctionType.Ln)
            rs = small_pool.tile([P, T], FP)
            nc.vector.reciprocal(rs[:], s_col[:])

            logp = small_pool.tile([P, T], FP)
            nc.vector.tensor_tensor(
                logp[:], xg[:], lse[:], op=mybir.AluOpType.subtract
            )
            pp = small_pool.tile([P, T], FP)  # p = exp(x)/s
            nc.vector.tensor_tensor(pp[:], ex[:], rs[:], op=mybir.AluOpType.mult)

            # u = (p-1)^2 * logp * alpha  ==  ((p-1)*logp) * ((p-1)*alpha)
            t1 = small_pool.tile([P, T], FP)
            nc.vector.scalar_tensor_tensor(
                t1[:],
                in0=pp[:],
                scalar=-1.0,
                in1=logp[:],
                op0=mybir.AluOpType.add,
                op1=mybir.AluOpType.mult,
            )
            t2 = small_pool.tile([P, T], FP)
            nc.vector.scalar_tensor_tensor(
                t2[:],
                in0=pp[:],
                scalar=-1.0,
                in1=alpha[:],
                op0=mybir.AluOpType.add,
                op1=mybir.AluOpType.mult,
            )
            u = small_pool.tile([P, T], FP)
            nc.vector.tensor_tensor(u[:], t1[:], t2[:], op=mybir.AluOpType.mult)

            # partial = sum_{p,t} u  (row-reduce on DVE, cross-partition on
            # GpSimd -- keeps the TensorEngine entirely out of the program)
            us = small_pool.tile([P, 1], FP)
            nc.vector.tensor_reduce(
                us[:], u[:], axis=mybir.AxisListType.X, op=mybir.AluOpType.add
            )
            fin_sb = small_pool.tile([1, 1], FP)
            nc.gpsimd.tensor_reduce(
                fin_sb[:], us[:], axis=mybir.AxisListType.C, op=mybir.AluOpType.add
            )
            nc.sync.dma_start(out.ap(), fin_sb[:])

            if dump_debug:
                nc.sync.dma_start(dbg["d_s"].ap(), s_col[:])
                nc.sync.dma_start(dbg["d_u"].ap(), u[:])
                nc.sync.dma_start(dbg["d_alpha"].ap(), alpha[:])

    nc.compile()
    return nc


_NC_CACHE = None


def _get_nc():
    global _NC_CACHE
    if _NC_CACHE is None:
        _NC_CACHE = build_program()
    return _NC_CACHE


def _to_bf16(a: np.ndarray) -> np.ndarray:
    """fp32 -> bf16 (round-to-nearest-even) as a uint16 view numpy array."""
    try:
        import ml_dtypes

        return a.astype(ml_dtypes.bfloat16)
    except ImportError:
        b = a.view(np.uint32)
        rounded = (b + 0x7FFF + ((b >> 16) & 1)) >> 16
        return rounded.astype(np.uint16).view(np.dtype("V2"))


def kernel(feature: np.ndarray, label: np.ndarray) -> np.ndarray:
    global LAST_RESULTS
    feature = np.ascontiguousarray(np.asarray(feature, dtype=np.float32))
    label = np.asarray(label)
    assert feature.shape == (N, C), feature.shape
    assert label.shape == (N,), label.shape

    lab32 = label.astype(np.int32)
    counts = np.bincount(lab32, minlength=C).astype(np.float32)  # global
    picked = feature[np.arange(N), lab32]  # label logits, fp32

    in_maps = []
    for k in range(N_CORES):
        fshard = feature[k * ROWS : (k + 1) * ROWS]
        lshard = lab32[k * ROWS : (k + 1) * ROWS]
        # column-major: [p, t] = row t*P + p, matching row-tile partitions
        lab_cm = lshard.reshape(T, P).T
        xg_cm = picked[k * ROWS : (k + 1) * ROWS].reshape(T, P).T
        in_maps.append(
            {
                "feature": np.ascontiguousarray(_to_bf16(fshard)),
                "xg_cm": np.ascontiguousarray(xg_cm),
                "cnt_cm": np.ascontiguousarray(counts[lab_cm]),
            }
        )

    nc = _get_nc()
    trace = bool(int(os.environ.get("KERNEL_TRACE", "0")))
    res = run_bass_kernel_spmd(
        nc,
        in_maps,
        core_ids=list(range(N_CORES)),
        trace=trace,
    )
    LAST_RESULTS = res

    total = 0.0
    for k in range(N_CORES):
        total += float(res.results[k]["out"][0, 0])
    return np.float32(-total / N)


# revision 18
# speedup vs baseline: 1.0686x; 1.0185x over previous
"""Cost-sensitive focal NLL loss on 8 Trainium2 NeuronCores.

Computes, for feature [N, C] logits and label [N] int:
    log_p = log_softmax(feature, axis=1)
    p = exp(log_p); beta = (1 - p)**2
    counts = bincount(label, C); ni = counts[label]; r = ni / N
    alpha = exp(r - 1) / r
    loss = -mean(alpha * beta[i, label[i]] * log_p[i, label[i]])

Only the label-column of log_p/beta is needed, so each core streams its
batch shard once (1 MiB DMAs, bf16 -- the 2e-2 tolerance dwarfs the
~2e-5 it costs), exponentiates on ScalarE and row-reduces on VectorE.
Everything that depends only on `label` (class histogram, gather
offsets -> the label logits themselves, per-row counts) is integer
bookkeeping prepared host-side during sharding; the device computes all
the float math: sumexp over [N, C], log-softmax at the label, focal
beta, cost weight alpha, and one partial sum per core.  The host adds
the 8 partials and divides by -N.
"""

import os

import numpy as np

import concourse.bacc as bacc
import concourse.mybir as mybir
import concourse.tile as tile
from concourse.bass_utils import run_bass_kernel_spmd

N_CORES = 8
N = 16384
C = 1000
P = 128
ROWS = N // N_CORES          # 2048 rows per core
T = ROWS // P                # 16 row-tiles per core
# row-tiles per DMA: small transfers first so ScalarE starts early, then
# uniform 0.5 MiB pairs that land faster than ScalarE consumes them (no
# mid-stream gaps from big-transfer completion latency).  Tile 14 sums via
# ScalarE accum_out to offload VectorE; the last tile reduces on VectorE so
# that reduce overlaps the Ln table load on ScalarE.
DMA_GROUPS = (1, 1, 2, 2, 2, 2, 2, 2, 1, 1)
ACCUM_TILES = (13, 14)       # row-sum via fused accum_out on ScalarE
assert sum(DMA_GROUPS) == T

FP = mybir.dt.float32
BF = mybir.dt.bfloat16

LAST_RESULTS = None  # BassKernelResults of the most recent run (for profiling)


def build_program(dump_debug: bool = False):
    nc = bacc.Bacc(
        "TRN2",
        target_bir_lowering=False,
        debug=False,
        enable_asserts=False,
        num_devices=N_CORES,
    )

    feature = nc.dram_tensor("feature", [ROWS, C], BF, kind="ExternalInput")
    xg_cm = nc.dram_tensor("xg_cm", [P, T], FP, kind="ExternalInput")
    cnt_cm = nc.dram_tensor("cnt_cm", [P, T], FP, kind="ExternalInput")
    out = nc.dram_tensor("out", [1, 1], FP, kind="ExternalOutput")
    dbg = {}
    if dump_debug:
        for nm in ("d_s", "d_u", "d_alpha"):
            dbg[nm] = nc.dram_tensor(nm, [P, T], FP, kind="ExternalOutput")

    with tile.TileContext(nc) as tc:
        with (
            tc.tile_pool(name="const", bufs=1) as const_pool,
            tc.tile_pool(name="feat", bufs=6) as feat_pool,
            tc.tile_pool(name="escr", bufs=4) as escr_pool,
            tc.tile_pool(name="small", bufs=1) as small_pool,
        ):
            neg1_col = const_pool.tile([P, 1], FP)
            nc.vector.memset(neg1_col[:], -1.0)

            # small inputs via the gpsimd SWDGE queue (idle during the whole
            # stream) so neither the sync queue nor ScalarE pays the dispatch
            xg = small_pool.tile([P, T], FP)
            nc.gpsimd.dma_start(xg[:], xg_cm.ap())
            cnt = small_pool.tile([P, T], FP)
            nc.gpsimd.dma_start(cnt[:], cnt_cm.ap())

            # ---- stream feature tiles: s[row] = sum_c exp(feature[row, c])
            # One ACT exp per landed DMA group.  Row-sums: per-tile [P, C]
            # reduces on VectorE for the leading groups, fused accum_out on
            # ScalarE for the trailing single-tile groups.
            feat_t = feature.ap().rearrange("(t p) c -> p t c", p=P)
            s_col = small_pool.tile([P, T], FP)
            t0 = 0
            for gi, g in enumerate(DMA_GROUPS):
                ft = feat_pool.tile([P, g * C], BF, name="ft")
                nc.sync.dma_start(
                    ft[:].rearrange("p (g c) -> p g c", g=g),
                    feat_t[:, t0 : t0 + g, :],
                )
                esc = escr_pool.tile([P, g * C], BF, name="esc")
                accum = t0 in ACCUM_TILES
                nc.scalar.activation(
                    esc[:],
                    ft[:],
                    mybir.ActivationFunctionType.Exp,
                    accum_out=s_col[:, t0 : t0 + 1] if accum else None,
                )
                if not accum:
                    for j in range(g):
                        nc.vector.tensor_reduce(
                            s_col[:, t0 + j : t0 + j + 1],
                            esc[:, j * C : (j + 1) * C],
                            axis=mybir.AxisListType.X,
                            op=mybir.AluOpType.add,
                        )
                t0 += g

                if gi == 2:
                    # per-row alpha = exp(r-1)/r, r = cnt/N (no Ln needed);
                    # emitted mid-stream: its inputs landed long ago, and this
                    # placement keeps the scheduler from parking it in front
                    # of the first stream exp on ScalarE
                    e1 = small_pool.tile([P, T], FP)
                    nc.scalar.activation(
                        e1[:],
                        cnt[:],
                        mybir.ActivationFunctionType.Exp,
                        bias=neg1_col[:],
                        scale=1.0 / N,
                    )
                    rc = small_pool.tile([P, T], FP)
                    nc.vector.reciprocal(rc[:], cnt[:])
                    alpha = small_pool.tile([P, T], FP)  # exp(r-1)*N*(1/cnt)
                    nc.vector.scalar_tensor_tensor(
                        alpha[:],
                        in0=e1[:],
                        scalar=float(N),
                        in1=rc[:],
                        op0=mybir.AluOpType.mult,
                        op1=mybir.AluOpType.mult,
                    )
                    ex = small_pool.tile([P, T], FP)
                    nc.scalar.activation(
                        ex[:], xg[:], mybir.ActivationFunctionType.Exp
                    )

            # ---- per-row tail ----
            lse = small_pool.tile([P, T], FP)
            nc.scalar.activation(lse[:], s_col[:], mybir.ActivationFunctionType.Ln)
            rs = small_pool.tile([P, T], FP)
            nc.vector.reciprocal(rs[:], s_col[:])

            logp = small_pool.tile([P, T], FP)
            nc.vector.tensor_tensor(
                logp[:], xg[:], lse[:], op=mybir.AluOpType.subtract
            )
            pp = small_pool.tile([P, T], FP)  # p = exp(x)/s
            nc.vector.tensor_tensor(pp[:], ex[:], rs[:], op=mybir.AluOpType.mult)

            # u = (p-1)^2 * logp * alpha  ==  ((p-1)*logp) * ((p-1)*alpha)
            t1 = small_pool.tile([P, T], FP)
            nc.vector.scalar_tensor_tensor(
                t1[:],
                in0=pp[:],
                scalar=-1.0,
                in1=logp[:],
                op0=mybir.AluOpType.add,
                op1=mybir.AluOpType.mult,
            )
            t2 = small_pool.tile([P, T], FP)
            nc.vector.scalar_tensor_tensor(
                t2[:],
                in0=pp[:],
                scalar=-1.0,
                in1=alpha[:],
                op0=mybir.AluOpType.add,
                op1=mybir.AluOpType.mult,
            )
            u = small_pool.tile([P, T], FP)
            nc.vector.tensor_tensor(u[:], t1[:], t2[:], op=mybir.AluOpType.mult)

            # partial = sum_{p,t} u  (row-reduce on DVE, cross-partition on
            # GpSimd -- keeps the TensorEngine entirely out of the program)
            us = small_pool.tile([P, 1], FP)
            nc.vector.tensor_reduce(
                us[:], u[:], axis=mybir.AxisListType.X, op=mybir.AluOpType.add
            )
            fin_sb = small_pool.tile([1, 1], FP)
            nc.gpsimd.tensor_reduce(
                fin_sb[:], us[:], axis=mybir.AxisListType.C, op=mybir.AluOpType.add
            )
            nc.sync.dma_start(out.ap(), fin_sb[:])

            if dump_debug:
                nc.sync.dma_start(dbg["d_s"].ap(), s_col[:])
                nc.sync.dma_start(dbg["d_u"].ap(), u[:])
                nc.sync.dma_start(dbg["d_alpha"].ap(), alpha[:])

    nc.compile()
    return nc


_NC_CACHE = None


def _get_nc():
    global _NC_CACHE
    if _NC_CACHE is None:
        _NC_CACHE = build_program()
    return _NC_CACHE


def _to_bf16(a: np.ndarray) -> np.ndarray:
    """fp32 -> bf16 (round-to-nearest-even) as a uint16 view numpy array."""
    try:
        import ml_dtypes

        return a.astype(ml_dtypes.bfloat16)
    except ImportError:
        b = a.view(np.uint32)
        rounded = (b + 0x7FFF + ((b >> 16) & 1)) >> 16
        return rounded.astype(np.uint16).view(np.dtype("V2"))


def kernel(feature: np.ndarray, label: np.ndarray) -> np.ndarray:
    global LAST_RESULTS
    feature = np.ascontiguousarray(np.asarray(feature, dtype=np.float32))
    label = np.asarray(label)
    assert feature.shape == (N, C), feature.shape
    assert label.shape == (N,), label.shape

    lab32 = label.astype(np.int32)
    counts = np.bincount(lab32, minlength=C).astype(np.float32)  # global
    picked = feature[np.arange(N), lab32]  # label logits, fp32

    in_maps = []
    for k in range(N_CORES):
        fshard = feature[k * ROWS : (k + 1) * ROWS]
        lshard = lab32[k * ROWS : (k + 1) * ROWS]
        # column-major: [p, t] = row t*P + p, matching row-tile partitions
        lab_cm = lshard.reshape(T, P).T
        xg_cm = picked[k * ROWS : (k + 1) * ROWS].reshape(T, P).T
        in_maps.append(
            {
                "feature": np.ascontiguousarray(_to_bf16(fshard)),
                "xg_cm": np.ascontiguousarray(xg_cm),
                "cnt_cm": np.ascontiguousarray(counts[lab_cm]),
            }
        )

    nc = _get_nc()
    trace = bool(int(os.environ.get("KERNEL_TRACE", "0")))
    res = run_bass_kernel_spmd(
        nc,
        in_maps,
        core_ids=list(range(N_CORES)),
        trace=trace,
    )
    LAST_RESULTS = res

    total = 0.0
    for k in range(N_CORES):
        total += float(res.results[k]["out"][0, 0])
    return np.float32(-total / N)
